# revision 41
# baseline (speedup 1.0000x reference)
"""FMM (dual-modality attention) Trainium2 kernel — v3.

Problem: b=4, c=256, w=h=64 (n=4096), float32.
  K/Kth/V/Vth = 1x1-conv projections of x / x_th.
  sim  = softmax_m(Kth^T K)            [n, n] per sample
  x_1  = V @ sim * gamma + x           (contraction over softmax ROWS)
  E    = Vth V^T (full n), sim_c = softmax_d(max_d E - E)   [c, c]
  x_2  = gamma2 * sim_c @ V + x_1

Sharding: 8 cores = (4 samples) x (2 halves of the n rows of sim).
Inputs per core are column-permuted so "my" half comes first; the host
un-permutes the P output columns for the second core of each pair.

v3 changes vs v2 (250us -> ~169us in the TimelineSim cost model):
  - P matmul in fp8e4 DoubleRow (2x128 contraction per pass, 0.5
    cyc/row): esim is re-encoded to fp8 as  e8 = esim * (128/rowsum),
    which by  exp(L-90) <= rowsum  is bounded by 128 (fp8e4 max 240)
    and >= 128/4096 for the dominant entry of the flattest row, i.e.
    always in fp8e4 normal range.  V rows go in as raw fp8; the
    1/128 and the softmax normalization both fold into the host-side
    gamma multiply.  One PSUM accumulation chain per (t, ch, m-chunk)
    over all 16 row-block pairs - no SBUF P staging adds.
  - all four projections consume one shared f32r copy of x / x_th
    (ACT rounding casts); the bf16 casting gpsimd DMAs are gone, so
    Pool is free to run its share of the fp8 re-encode scans.
  - the Vn/Vthn projections, simcT transposes and A (channel
    attention) matmuls are emitted as PE fillers inside the ACT-bound
    sim phase (Vn re-streams x as bf16 via casting DMAs, so the f32r
    copies of x can die with phase 1); A output is written bf16.
  - rescales of the last blocks are split across DVE+Pool so the P
    DoubleRow chains (which need every esim pair) start sooner.
"""

import os
import sys

sys.path.insert(0, "/opt/trn_rl_repo")

_ABLATE = set(os.environ.get("BASS_ABLATE", "").split(","))

import numpy as np
import ml_dtypes

import concourse.bass as bass
import concourse.bacc as bacc
import concourse.tile as tile
from concourse import mybir
from concourse.bass_utils import run_bass_kernel_spmd

F32 = mybir.dt.float32
F32R = mybir.dt.float32r
BF16 = mybir.dt.bfloat16
FP8 = mybir.dt.float8e4

C = 256          # channels
N = 4096         # w*h
HALF = N // 2    # rows per core
P = 128          # partitions
CH = C // P      # channel halves (2)
MC = 512         # m chunk (one PSUM bank of f32)
NMC = N // MC    # 8 chunks over full m
NMC_H = HALF // MC   # 4 chunks over half
HBLK = HALF // P     # 16 row-blocks per core
NPAIR = HBLK // 2    # 8 block pairs for DoubleRow
WCOLS = 2048 + 8 + 2 * C  # packed const buffer columns
EXP_BIAS = -90.0  # fixed softmax shift (see module docstring)
CF = 128.0        # fp8 esim cap: e8 = esim * CF / rowsum


def build_program(g_rgb, g_th, g2_rgb, g2_th):
    nc = bacc.Bacc("TRN2", target_bir_lowering=False, debug=False, num_devices=8)

    def din(name, shape):
        return nc.dram_tensor(name, shape, F32, kind="ExternalInput").ap()

    x_d = din("x", [C, N])       # my-half-first permuted x
    xth_d = din("xth", [C, N])   # my-half-first permuted x_th
    wconst_d = din("wconst", [P, WCOLS])

    P_d = nc.dram_tensor("P_out", [2, C, N], BF16, kind="ExternalOutput").ap()
    A_d = nc.dram_tensor("A_out", [C, HALF], BF16, kind="ExternalOutput").ap()
    Ath_d = nc.dram_tensor("Ath_out", [C, HALF], BF16,
                           kind="ExternalOutput").ap()

    with tile.TileContext(nc) as tc:
        _build_tile(tc, nc, x_d, xth_d, wconst_d, P_d, A_d, Ath_d,
                    g_rgb, g_th, g2_rgb, g2_th)
    nc.finalize()
    return nc


def _build_tile(tc, nc, x_d, xth_d, wconst_d, P_d, A_d, Ath_d,
                g_rgb, g_th, g2_rgb, g2_th):
    from contextlib import ExitStack

    Ident = mybir.ActivationFunctionType.Identity
    Exp = mybir.ActivationFunctionType.Exp
    Alu = mybir.AluOpType
    AX = mybir.AxisListType
    DR = mybir.MatmulPerfMode.DoubleRow

    ctx = ExitStack()
    with ctx:
        # ---- persistent projection outputs (live into phase 3) ----
        proj = ctx.enter_context(tc.tile_pool(name="proj", bufs=1))
        K_sb = proj.tile([P, CH, N], F32R, tag="K")          # 32KB/part
        Kth_sb = proj.tile([P, CH, HALF], F32R, tag="Kth")   # 16KB/part
        VT8 = proj.tile([P, HBLK, C], FP8, tag="VT8")        # 4KB/part
        VthT8 = proj.tile([P, HBLK, C], FP8, tag="VthT8")
        Vn_sb = proj.tile([P, CH, HALF], BF16, tag="Vn")     # 8KB/part
        Vthn_sb = proj.tile([P, CH, HALF], BF16, tag="Vthn")
        ebias = proj.tile([P, 1], F32, tag="ebias")
        nc.vector.memset(ebias[:], EXP_BIAS)
        simcT = proj.tile([P, 2, CH, C], BF16, tag="simcT")
        wv_bf = proj.tile([P, 1024], BF16, tag="wv_bf")   # phase-3 Vn weights
        bvp = proj.tile([P, 4], F32, tag="bvp")           # bv | bvth columns
        simc = proj.tile([P, CH, C], F32, tag="simc")
        identp = proj.tile([P, P], F32, tag="identp")

        # A staging lives in the outer scope so inner pool closes never
        # wait on the A output DMAs
        ctr = ctx.enter_context(tc.tile_pool(name="chan_tr", bufs=2))

        # energy PSUM accumulates across the whole V^T loop (32 blocks);
        # one bank per channel half
        epsum_cm = tc.tile_pool(name="e_psum", bufs=2, space="PSUM", side="right")
        epsum = epsum_cm.__enter__()
        eps = []
        for _ch in range(CH):
            eps_t = epsum.tile([P, C], F32, tag="eps")
            eps.append(eps_t[:])

        with ExitStack() as cctx:   # consts + phase-1/2 transients
            const = cctx.enter_context(tc.tile_pool(name="const", bufs=1))
            const_sb = const.tile([P, WCOLS], F32, tag="wconst")
            for piece in range(4):
                lo = 1024 + piece * 256
                nc.scalar.dma_start(out=const_sb[:, lo:lo + 256],
                                    in_=wconst_d[:, lo:lo + 256])
            # all weights as f32r (single rounding-cast each); wv first:
            # they gate the opening V^T matmuls
            wv_r = const.tile([P, 1024], F32R, tag="wv_r")
            for piece in range(4):
                nc.scalar.activation(
                    out=wv_r[:, piece * 256:(piece + 1) * 256],
                    in_=const_sb[:, 1024 + piece * 256:1024 + (piece + 1) * 256],
                    func=Ident, bias=0.0)
            wk_r = const.tile([P, 1024], F32R, tag="wk_r")

            def wk_lhsT(ci, co):      # Wk_rgb^T slice [c_in 128, c_out 128]
                return wk_r[:, ci * C + co * P: ci * C + co * P + P]

            def wkth_lhsT(ci, co):
                return wk_r[:, 512 + ci * C + co * P: 512 + ci * C + co * P + P]

            def wv_lhsT(ci, co):
                return wv_r[:, ci * C + co * P: ci * C + co * P + P]

            def wvth_lhsT(ci, co):
                return wv_r[:, 512 + ci * C + co * P: 512 + ci * C + co * P + P]

            def wv_rhs(ci):           # [c_in 128, c_out 256] moving operand
                return wv_r[:, ci * C: (ci + 1) * C]

            def wvth_rhs(ci):
                return wv_r[:, 512 + ci * C: 512 + (ci + 1) * C]

            BOFF = 2048
            bk = const_sb[:, BOFF + 0: BOFF + 2]
            bkth = const_sb[:, BOFF + 2: BOFF + 4]
            bv = const_sb[:, BOFF + 4: BOFF + 6]
            bvth = const_sb[:, BOFF + 6: BOFF + 8]
            bvb = const_sb[:, BOFF + 8: BOFF + 8 + C]
            bvthb = const_sb[:, BOFF + 8 + C: BOFF + 8 + 2 * C]

            identity = const.tile([P, P], F32, tag="ident")
            from concourse.masks import make_identity
            make_identity(nc, identity[:])

            # ====== phase 1+2: projections, energy, channel attention ======
            with ExitStack() as pctx:
                xrp = pctx.enter_context(tc.tile_pool(name="xr", bufs=1))
                # shared f32r copies of x / x_th (all projections read these)
                xr = xrp.tile([P, CH, N], F32R, tag="xr")      # 32KB/part
                xthr = xrp.tile([P, CH, N], F32R, tag="xthr")  # 32KB/part
                xst = pctx.enter_context(tc.tile_pool(name="xstage", bufs=3))
                vtt = pctx.enter_context(tc.tile_pool(name="vt_tmp", bufs=4))
                vbf = pctx.enter_context(tc.tile_pool(name="vt_bf", bufs=1))
                cpool = pctx.enter_context(tc.tile_pool(name="chan", bufs=1))
                ppsum = pctx.enter_context(
                    tc.tile_pool(name="proj_psum", bufs=3, space="PSUM"))
                vpsum = pctx.enter_context(
                    tc.tile_pool(name="vt_psum", bufs=3, space="PSUM"))

                # bf16 V^T tiles (energy path), phase-1/2 scoped
                VT_sb = vbf.tile([P, HBLK, C], BF16, tag="VT")
                VthT_sb = vbf.tile([P, HBLK, C], BF16, tag="VthT")

                # stage f32 chunks and round-cast to f32r. my halves first,
                # in fine chunks so the V^T loop starts early.
                def stage(dst, src_d, c0, q, mc=MC):
                    st = xst.tile([P, CH, mc], F32, tag="xs", name="xs")
                    for ci in range(CH):
                        q.dma_start(out=st[:, ci, :],
                                    in_=src_d[ci * P:(ci + 1) * P, c0:c0 + mc])
                    nc.gpsimd.tensor_copy(out=dst[:, :, c0:c0 + mc], in_=st[:])

                # interleave x/xth quarters in V^T consumption order; the
                # first pieces are split fine so the V^T loop starts early
                for ci in range(CH):
                    nc.gpsimd.dma_start(out=xr[:, ci, 0:MC],
                                        in_=x_d[ci * P:(ci + 1) * P, 0:MC])
                nc.sync.dma_start(out=const_sb[:, 2048:], in_=wconst_d[:, 2048:])
                for ci in range(CH):
                    nc.gpsimd.dma_start(out=xthr[:, ci, 0:MC],
                                        in_=xth_d[ci * P:(ci + 1) * P, 0:MC])
                for q4 in range(1, NMC_H):
                    stage(xr, x_d, q4 * MC, nc.sync)
                    stage(xthr, xth_d, q4 * MC, nc.scalar)
                nc.sync.dma_start(out=const_sb[:, 0:1024],
                                  in_=wconst_d[:, 0:1024])
                for q4 in range(NMC_H):
                    stage(xr, x_d, HALF + q4 * MC, nc.sync)
                    stage(xthr, xth_d, HALF + q4 * MC, nc.scalar)

                # V^T / Vth^T my blocks (kept, + fp8 copies) and other
                # blocks (transient), energy accumulated over all 32 blocks
                estop = 2 * HBLK - 1
                for blk in range(2 * HBLK):
                    oth = blk >= HBLK
                    j = blk - HBLK if oth else blk
                    xoff = HALF if oth else 0
                    if oth:
                        vt_v = vtt.tile([P, C], BF16, tag="vt_o")
                        vt_t = vtt.tile([P, C], BF16, tag="vtt_o")
                        for (xb, wrhs, bbc, dstt) in (
                                (xr, wv_rhs, bvb, vt_v),
                                (xthr, wvth_rhs, bvthb, vt_t)):
                            ps = vpsum.tile([P, C], F32, tag="ppv")
                            for ci in range(CH):
                                nc.tensor.matmul(
                                    ps[:],
                                    xb[:, ci, xoff + j * P:xoff + (j + 1) * P],
                                    wrhs(ci),
                                    start=(ci == 0), stop=(ci == CH - 1))
                            nc.vector.tensor_add(dstt[:], ps[:], bbc[:])
                        for ch in range(CH):
                            nc.tensor.matmul(eps[ch],
                                             vt_t[:, ch * P:(ch + 1) * P], vt_v[:],
                                             start=(blk == 0), stop=(blk == estop))
                    else:
                        for (xb, wrhs, bbc, dtile) in (
                                (xr, wv_rhs, bvb, VT_sb),
                                (xthr, wvth_rhs, bvthb, VthT_sb)):
                            ps = vpsum.tile([P, C], F32, tag="ppv")
                            for ci in range(CH):
                                nc.tensor.matmul(
                                    ps[:], xb[:, ci, j * P:(j + 1) * P], wrhs(ci),
                                    start=(ci == 0), stop=(ci == CH - 1))
                            nc.vector.tensor_add(dtile[:, j, :], ps[:], bbc[:])
                        for ch in range(CH):
                            nc.tensor.matmul(eps[ch],
                                             VthT_sb[:, j, ch * P:(ch + 1) * P],
                                             VT_sb[:, j, :],
                                             start=(blk == 0), stop=(blk == estop))

                # wk cast deferred here: the [0:1024] const DMA lands
                # mid-stream and an early ACT wait would block the queue
                nc.scalar.activation(out=wk_r[:], in_=const_sb[:, 0:1024],
                                     func=Ident, bias=0.0)

                # fp8 copies of V^T for the DoubleRow P matmul (ACT,
                # deferred here so they fill the K-loop window)
                for j in range(HBLK):
                    nc.scalar.activation(out=VT8[:, j, :], in_=VT_sb[:, j, :],
                                         func=Ident, bias=0.0)
                    nc.scalar.activation(out=VthT8[:, j, :], in_=VthT_sb[:, j, :],
                                         func=Ident, bias=0.0)

                # channel softmax (ACT/DVE; overlaps the projection matmuls
                # below): sim_c = softmax_d(minE[c] - E[c, d])
                E_sb = cpool.tile([P, CH, C], F32, tag="E")
                for ch in range(CH):
                    nc.vector.tensor_copy(out=E_sb[:, ch, :], in_=eps[ch])
                epsum_cm.__exit__(None, None, None)

                emin = cpool.tile([P, CH], F32, tag="emin")
                esum = cpool.tile([P, CH], F32, tag="esum")
                einv = cpool.tile([P, CH], F32, tag="einv")
                for ch in range(CH):
                    nc.vector.tensor_reduce(out=emin[:, ch:ch + 1], in_=E_sb[:, ch, :],
                                            axis=AX.X, op=Alu.min)
                    nc.scalar.activation(out=simc[:, ch, :], in_=E_sb[:, ch, :],
                                         func=Exp, bias=emin[:, ch:ch + 1],
                                         scale=-1.0, accum_out=esum[:, ch:ch + 1])
                    nc.vector.reciprocal(out=einv[:, ch:ch + 1], in_=esum[:, ch:ch + 1])
                    nc.vector.tensor_scalar_mul(simc[:, ch, :], simc[:, ch, :],
                                                einv[:, ch:ch + 1])

                # PSUM drains split DVE-heavy (ACT is exp-loaded in phase 3)
                def drain_bias(k, out, ps, bias):
                    if k % 2 == 0:
                        nc.scalar.activation(out=out, in_=ps[:], func=Ident,
                                             bias=bias)
                    else:
                        nc.vector.tensor_scalar_add(out, ps[:], bias)

                # K natural [c_out, m] over full (permuted) m (f32r logits)
                for mc in range(NMC):
                    off = mc * MC
                    for co in range(CH):
                        ps = ppsum.tile([P, MC], F32, tag="pp")
                        for ci in range(CH):
                            nc.tensor.matmul(
                                ps[:], wk_lhsT(ci, co), xr[:, ci, off:off + MC],
                                start=(ci == 0), stop=(ci == CH - 1))
                        drain_bias(mc * CH + co, K_sb[:, co, off:off + MC],
                                   ps, bk[:, co:co + 1])

                # Kth [c_out, i(my rows)] from xth my half
                for mc in range(NMC_H):
                    off = mc * MC
                    for co in range(CH):
                        ps = ppsum.tile([P, MC], F32, tag="pp")
                        for ci in range(CH):
                            nc.tensor.matmul(
                                ps[:], wkth_lhsT(ci, co), xthr[:, ci, off:off + MC],
                                start=(ci == 0), stop=(ci == CH - 1))
                        drain_bias(mc * CH + co,
                                   Kth_sb[:, co, off:off + MC],
                                   ps, bkth[:, co:co + 1])

                # Vn/Vthn and the simcT transposes move into phase 3 as PE
                # fillers; persist the small operands they need
                nc.vector.tensor_copy(out=wv_bf[:], in_=const_sb[:, 1024:2048])
                nc.vector.tensor_copy(out=bvp[:], in_=const_sb[:, BOFF + 4:BOFF + 8])
                nc.vector.tensor_copy(out=identp[:], in_=identity[:])

        # =============== phase 3: spatial attention ===============
        MCE = 1024          # exp chunk (2 PSUM banks)
        NMCE = N // MCE     # 4
        with ExitStack() as sctx:
            epool = sctx.enter_context(tc.tile_pool(name="expsim", bufs=4))
            e8pool = sctx.enter_context(tc.tile_pool(name="e8", bufs=NPAIR))
            stat = sctx.enter_context(tc.tile_pool(name="stat", bufs=4))
            pstage = sctx.enter_context(tc.tile_pool(name="pstage", bufs=4))
            spsum = sctx.enter_context(
                tc.tile_pool(name="s_psum", bufs=2, space="PSUM"))
            ppsum2 = sctx.enter_context(
                tc.tile_pool(name="p_psum", bufs=4, space="PSUM"))

            e8s = [e8pool.tile([P, 2, N], FP8, tag="e8", name=f"e8_{i}")
                   for i in range(NPAIR)]

            def emit_sim_block(blk):
                esim = epool.tile([P, N], BF16, tag="esim")
                rs = stat.tile([P, NMCE], F32, tag="rs")
                for mce in range(NMCE):
                    ps = spsum.tile([P, MCE], F32, tag="sps")
                    if "smm" not in _ABLATE:
                        for half in range(2):
                            mc = 2 * mce + half
                            for ci in range(CH):
                                nc.tensor.matmul(
                                    ps[:, half * MC:(half + 1) * MC],
                                    Kth_sb[:, ci, blk * P:(blk + 1) * P],
                                    K_sb[:, ci, mc * MC:(mc + 1) * MC],
                                    start=(ci == 0), stop=(ci == CH - 1))
                    if "exp" not in _ABLATE:
                        nc.scalar.activation(
                            out=esim[:, mce * MCE:(mce + 1) * MCE],
                            in_=ps[:], func=Exp, bias=ebias[:],
                            accum_out=rs[:, mce:mce + 1])
                return esim, rs

            def emit_stats_rescale(blk, esim, rs):
                # f = CF / rowsum, then e8 = esim * f (fp8, DoubleRow pair
                # layout). Rescale alternates DVE / Pool.
                rowsum = stat.tile([P, 1], F32, tag="rowsum")
                nc.vector.tensor_reduce(out=rowsum[:], in_=rs[:], axis=AX.X,
                                        op=Alu.add)
                rinv = stat.tile([P, 1], F32, tag="rinv")
                nc.vector.reciprocal(out=rinv[:], in_=rowsum[:])
                f = stat.tile([P, 1], F32, tag="f")
                nc.vector.tensor_scalar_mul(f[:], rinv[:], CF)
                dst = e8s[blk // 2][:, blk % 2, :]
                if "resc" in _ABLATE:
                    return
                if blk >= 14:
                    QS = N // 4
                    for qq in range(4):
                        eng = nc.vector if qq % 2 == 0 else nc.gpsimd
                        eng.tensor_scalar_mul(dst[:, qq * QS:(qq + 1) * QS],
                                              esim[:, qq * QS:(qq + 1) * QS],
                                              f[:])
                elif blk == 13:
                    nc.vector.tensor_scalar_mul(dst[:, 0:HALF], esim[:, 0:HALF],
                                                f[:])
                    nc.gpsimd.tensor_scalar_mul(dst[:, HALF:], esim[:, HALF:],
                                                f[:])
                else:
                    eng = nc.gpsimd if blk in (2, 5, 8, 11) else nc.vector
                    eng.tensor_scalar_mul(dst, esim[:], f[:])

            def emit_A_group(grp):
                # one (t, ch) group of the channel attention:
                # A = gamma2 * (simc @ V), residual added on host
                t, ch = divmod(grp, CH)
                vn = Vn_sb if t == 0 else Vthn_sb
                out_d = A_d if t == 0 else Ath_d
                HC = 1024
                for hh in range(HALF // HC):
                    ast = ctr.tile([P, HC], BF16, tag="astage")
                    for m2 in range(HC // MC):
                        mc = hh * (HC // MC) + m2
                        ps = ppsum2.tile([P, MC], F32, tag="ppp")
                        for dh in range(CH):
                            nc.tensor.matmul(
                                ps[:], simcT[:, t, dh, ch * P:(ch + 1) * P],
                                vn[:, dh, mc * MC:(mc + 1) * MC],
                                start=(dh == 0), stop=(dh == CH - 1))
                        nc.vector.tensor_copy(
                            out=ast[:, m2 * MC:(m2 + 1) * MC], in_=ps[:])
                    q = nc.sync if hh % 2 == 0 else nc.scalar
                    q.dma_start(
                        out=out_d[ch * P:(ch + 1) * P, hh * HC:(hh + 1) * HC],
                        in_=ast[:])

            def emit_P_group(grp):
                # one (t, ch) group: 8 m-chunks, each a single PSUM chain of
                # 8 DoubleRow pair-matmuls over all 16 row blocks
                if "pmm" in _ABLATE:
                    return
                t, ch = divmod(grp, CH)
                V8 = VT8 if t == 0 else VthT8
                for mc in range(NMC):
                    ps = ppsum2.tile([P, MC], F32, tag="ppp")
                    for bp in range(NPAIR):
                        nc.tensor.matmul(
                            ps[:],
                            V8[:, 2 * bp:2 * bp + 2, ch * P:(ch + 1) * P],
                            e8s[bp][:, :, mc * MC:(mc + 1) * MC],
                            start=(bp == 0), stop=(bp == NPAIR - 1),
                            perf_mode=DR)
                    pst = pstage.tile([P, MC], BF16, tag="pst")
                    if mc % 2 == 0:
                        nc.scalar.activation(out=pst[:], in_=ps[:], func=Ident,
                                             bias=0.0)
                    else:
                        nc.vector.tensor_copy(out=pst[:], in_=ps[:])
                    q = nc.sync if mc % 2 == 0 else nc.scalar
                    q.dma_start(
                        out=P_d[t, ch * P:(ch + 1) * P, mc * MC:(mc + 1) * MC],
                        in_=pst[:])

            vnst = sctx.enter_context(tc.tile_pool(name="vnst", bufs=3))

            def wvbf_lhsT(t, ci, co):
                return wv_bf[:, t * 512 + ci * C + co * P:
                             t * 512 + ci * C + co * P + P]

            def emit_vn_chunk(mc):
                # Vn/Vthn projection chunk from a fresh bf16 casting-DMA
                # stream (HWDGE is idle during the sim phase)
                off = mc * MC
                for t, (src_d, dst) in enumerate(((x_d, Vn_sb),
                                                  (xth_d, Vthn_sb))):
                    xb = vnst.tile([P, CH, MC], BF16, tag="vn_x", name="vn_x")
                    for ci in range(CH):
                        nc.gpsimd.dma_start(
                            out=xb[:, ci, :],
                            in_=src_d[ci * P:(ci + 1) * P, off:off + MC])
                    for co in range(CH):
                        ps = ppsum2.tile([P, MC], F32, tag="ppp", name="ppp")
                        for ci in range(CH):
                            nc.tensor.matmul(
                                ps[:], wvbf_lhsT(t, ci, co), xb[:, ci, :],
                                start=(ci == 0), stop=(ci == CH - 1))
                        if (mc + co) % 2 == 0:
                            nc.scalar.activation(
                                out=dst[:, co, off:off + MC], in_=ps[:],
                                func=Ident, bias=bvp[:, 2 * t + co:2 * t + co + 1])
                        else:
                            nc.vector.tensor_scalar_add(
                                dst[:, co, off:off + MC], ps[:],
                                bvp[:, 2 * t + co:2 * t + co + 1])

            def emit_simcT():
                for a in range(CH):
                    for bnk in range(CH):
                        tps_t = ppsum2.tile([P, MC], F32, tag="ppp",
                                            name="ppp")
                        tps = tps_t[:, 0:P]
                        nc.tensor.transpose(
                            tps, simc[:, a, bnk * P:(bnk + 1) * P], identp[:])
                        nc.vector.tensor_scalar_mul(
                            simcT[:, 0, bnk, a * P:(a + 1) * P], tps, g2_rgb)
                        nc.vector.tensor_scalar_mul(
                            simcT[:, 1, bnk, a * P:(a + 1) * P], tps, g2_th)

            VN_AT = {0: 0, 2: 1, 4: 2, 6: 3}
            A_AT = {8: 0, 10: 1, 12: 2, 15: 3}
            for blk in range(HBLK):
                esim, rs = emit_sim_block(blk)
                if blk in VN_AT:
                    emit_vn_chunk(VN_AT[blk])
                elif blk == 7:
                    emit_simcT()
                if blk in A_AT:
                    emit_A_group(A_AT[blk])
                emit_stats_rescale(blk, esim, rs)
            for grp in range(4):
                emit_P_group(grp)


_PROGRAM_CACHE = {}
_LAST_IN_MAPS = None


def kernel(**inputs):
    global _LAST_IN_MAPS
    x = np.ascontiguousarray(inputs["x"], dtype=np.float32)        # [4, 256, 64, 64]
    x_th = np.ascontiguousarray(inputs["x_th"], dtype=np.float32)
    b = x.shape[0]
    gammas = tuple(float(np.asarray(inputs[k]).reshape(-1)[0])
                   for k in ("gamma_rgb", "gamma_th", "gamma2_rgb", "gamma2_th"))

    if gammas not in _PROGRAM_CACHE:
        _PROGRAM_CACHE[gammas] = build_program(*gammas)
    nc = _PROGRAM_CACHE[gammas]

    # packed const buffer [128, WCOLS]:
    #   [0:512] wkT, [512:1024] wkthT, [1024:1536] wvT, [1536:2048] wvthT
    #     (col = ci*256 + c_out; row p = c_in within half ci)
    #   [2048:2056] biases bk|bkth|bv|bvth (2 cols each, col h -> b[h*128+p])
    #   [2056:2568] bvb | bvthb broadcast rows (row p, col c -> b[c])
    wconst = np.zeros((P, WCOLS), np.float32)

    def pack_w(dst_off, W):
        wt = np.asarray(W, np.float32).T.reshape(CH, P, C).transpose(1, 0, 2)
        wconst[:, dst_off:dst_off + 512] = wt.reshape(P, 512)

    pack_w(0, inputs["Wk_rgb"])
    pack_w(512, inputs["Wk_th"])
    pack_w(1024, inputs["Wv_rgb"])
    pack_w(1536, inputs["Wv_th"])
    for t, key in enumerate(("bk_rgb", "bk_th", "bv_rgb", "bv_th")):
        bias = np.asarray(inputs[key], np.float32)
        wconst[:, 2048 + 2 * t: 2048 + 2 * t + 2] = bias.reshape(CH, P).T
    wconst[:, 2056:2056 + C] = np.asarray(inputs["bv_rgb"], np.float32)[None, :]
    wconst[:, 2056 + C:2056 + 2 * C] = np.asarray(inputs["bv_th"], np.float32)[None, :]

    in_maps = []
    for k in range(8):
        s, hh = divmod(k, 2)
        xs = x[s].reshape(C, N)
        xths = x_th[s].reshape(C, N)
        if hh == 0:
            xp, xthp = xs, xths
        else:
            xp = np.concatenate([xs[:, HALF:], xs[:, :HALF]], axis=1)
            xthp = np.concatenate([xths[:, HALF:], xths[:, :HALF]], axis=1)
        in_maps.append({
            "x": np.ascontiguousarray(xp),
            "xth": np.ascontiguousarray(xthp),
            "wconst": wconst,
        })

    _LAST_IN_MAPS = (nc, in_maps)
    res = run_bass_kernel_spmd(nc, in_maps, core_ids=list(range(8))).results

    g_rgb, g_th = gammas[0], gammas[1]
    out = np.empty((b, C, N), np.float32)
    out_th = np.empty((b, C, N), np.float32)
    for s in range(b):
        k0, k1 = 2 * s, 2 * s + 1
        for (dst, a_key, t, gam, xs) in ((out, "A_out", 0, g_rgb, x),
                                         (out_th, "Ath_out", 1, g_th, x_th)):
            acc = np.concatenate([res[k0][a_key], res[k1][a_key]],
                                 axis=1).astype(np.float32)
            P0 = res[k0]["P_out"][t].astype(np.float32)
            P1 = res[k1]["P_out"][t].astype(np.float32)
            P1 = np.concatenate([P1[:, HALF:], P1[:, :HALF]], axis=1)
            dst[s] = acc + (gam / CF) * (P0 + P1) + xs[s].reshape(C, N)

    w = int(np.sqrt(N))
    return out.reshape(b, C, w, w), out_th.reshape(b, C, w, w)


def timed_run(inputs=None):
    """Re-run the last compiled program with NTFF profiling; return exec ns."""
    if _LAST_IN_MAPS is None:
        if inputs is not None:
            kernel(**inputs)
        else:
            return None
    nc, in_maps = _LAST_IN_MAPS
    try:
        r = run_bass_kernel_spmd(nc, in_maps, core_ids=list(range(8)), trace=True)
        return r.exec_time_ns
    except Exception as e:  # profiling infra may be unavailable
        print(f"timed_run trace failed: {e}")
        return None
